# revision 8
# baseline (speedup 1.0000x reference)
"""v6: attention head on 8 trn2 NeuronCores, no collectives.

Sharding: core c handles batch b=c//2 and K/V-half j=c%2. Each core computes
unnormalized attention of the batch's FULL query block (2048 rows) against its
1024-row K/V half; softmax numerator/denominator halves combine linearly on
the host (out = (numA+numB)/(denA+denB)).

Host-side preprocessing (layout/dtype only, no model FLOPs): q/k/v cast to
bf16 and pre-transposed to [H, seq]; the 1/8 score scale folded into Wq/bq.

Device per core: chunked HWDGE loads (w, kT, qn0, qn1, vT, qn2, qn3) so
projections stream behind the DMA; scores in [k, q] layout; exp on ACT
(PSUM->SBUF bf16); attnV with V-natural stationary carrying a ones column
(row 64 = denominator); per-q-half epilogue transposes the [65, q] result to
q-natural [128, 520] so the output DMA uses all 128 partitions.
"""

import sys

if "/opt/trn_rl_repo" not in sys.path:
    sys.path.insert(0, "/opt/trn_rl_repo")

import numpy as np
import ml_dtypes

N, L, H, D = 4, 2048, 1024, 64
NCORES = 8
HC = H // 128  # 8 h-chunks
KH = L // 2  # 1024 rows of K/V per core
KC = KH // 128  # 8 k-chunks per core


def build_bass():
    import concourse.mybir as mybir
    from concourse import bacc
    from concourse.masks import make_identity
    from concourse.tile import TileContext

    f32 = mybir.dt.float32
    bf16 = mybir.dt.bfloat16
    AF = mybir.ActivationFunctionType

    nc = bacc.Bacc("TRN2", target_bir_lowering=False, debug=False)
    qt_d = nc.dram_tensor("qt", [128, 16 * H], bf16, kind="ExternalInput").ap()
    kt_d = nc.dram_tensor("kt", [128, 8 * H], bf16, kind="ExternalInput").ap()
    vt_d = nc.dram_tensor("vt", [128, 8 * H], bf16, kind="ExternalInput").ap()
    w_d = nc.dram_tensor("w", [128, 3 * HC * D], bf16, kind="ExternalInput").ap()
    b_d = nc.dram_tensor("b", [D, 3], f32, kind="ExternalInput").ap()
    out_d = nc.dram_tensor("out", [128, 16 * (D + 1)], f32, kind="ExternalOutput").ap()

    with TileContext(nc) as tc:
        with (
            tc.tile_pool(name="io", bufs=1) as io_pool,
            tc.tile_pool(name="proj", bufs=1) as proj_pool,
            tc.tile_pool(name="e", bufs=8) as e_pool,
            tc.tile_pool(name="ps", bufs=2, space="PSUM") as ps_pool,
            tc.tile_pool(name="acc", bufs=1, space="PSUM") as acc_pool,
        ):
            w_sb = io_pool.tile([128, 3 * HC * D], bf16, tag="w")
            b_sb = io_pool.tile([D, 3], f32, tag="b")
            kt_sb = io_pool.tile([128, 8 * H], bf16, tag="kt")
            vt_sb = io_pool.tile([128, 8 * H], bf16, tag="vt")
            qt_sb = io_pool.tile([128, 16 * H], bf16, tag="qt")
            # DMA issue order defines HWDGE FIFO: w, kT halves, qn0, qn1,
            # vT halves, qn2, qn3
            nc.sync.dma_start(out=w_sb[:], in_=w_d[:])
            nc.sync.dma_start(out=b_sb[:], in_=b_d[:])
            for i in range(2):
                nc.sync.dma_start(
                    out=kt_sb[:, i * 4 * KH : (i + 1) * 4 * KH],
                    in_=kt_d[:, i * 4 * KH : (i + 1) * 4 * KH],
                )
            for qn in range(2):
                nc.sync.dma_start(
                    out=qt_sb[:, qn * 4 * H : (qn + 1) * 4 * H],
                    in_=qt_d[:, qn * 4 * H : (qn + 1) * 4 * H],
                )
            for i in range(2):
                nc.sync.dma_start(
                    out=vt_sb[:, i * 4 * KH : (i + 1) * 4 * KH],
                    in_=vt_d[:, i * 4 * KH : (i + 1) * 4 * KH],
                )
            for qn in range(2, 4):
                nc.sync.dma_start(
                    out=qt_sb[:, qn * 4 * H : (qn + 1) * 4 * H],
                    in_=qt_d[:, qn * 4 * H : (qn + 1) * 4 * H],
                )

            identf = io_pool.tile([128, 128], f32, tag="identf")
            make_identity(nc, identf[:])

            kprojT = proj_pool.tile([D, KH], bf16, tag="kprojT")
            qprojT = proj_pool.tile([D, L], bf16, tag="qprojT")
            vp = proj_pool.tile([128, KC * (D + 1)], bf16, tag="vp")
            outT_sb = proj_pool.tile([D + 1, L], f32, tag="outT")
            out_sb = proj_pool.tile([128, 16 * (D + 1)], f32, tag="out")

            def proj(ps, wbase, xt_sb, base, hstride, ncols):
                """ps[0:64, 0:ncols] += sum_hc W[hc].T @ xT[:, base + hc*hstride ...]"""
                for hc in range(HC):
                    wslice = w_sb[:, (wbase + hc) * D : (wbase + hc + 1) * D]
                    for sn in range(ncols // 512):
                        c0 = base + hc * hstride + sn * 512
                        nc.tensor.matmul(
                            ps[0:D, sn * 512 : (sn + 1) * 512],
                            wslice,
                            xt_sb[:, c0 : c0 + 512],
                            start=(hc == 0), stop=(hc == HC - 1),
                        )

            # ---- K projection ----
            psk = ps_pool.tile([128, 1024], f32, tag="ps", name="psk")
            proj(psk, HC, kt_sb, 0, KH, 1024)
            nc.vector.tensor_scalar_add(kprojT[:], psk[0:D, :], b_sb[:, 1:2])

            # ---- Q projection, first half (qn0, qn1) ----
            for qnp, qn in ((0, 0), (0, 1)):
                psq = ps_pool.tile([128, 1024], f32, tag="ps", name=f"psq{qn}")
                proj(psq, 0, qt_sb, qn * 4 * H, 512, 512)
                nc.vector.tensor_scalar_add(
                    qprojT[:, qn * 512 : (qn + 1) * 512], psq[0:D, 0:512],
                    b_sb[:, 0:1],
                )

            # ---- scores + exp for q-half 0 ----
            e_tiles = {}
            for kc in range(KC):
                sct = ps_pool.tile([128, 1024], f32, tag="ps", name=f"sc0_{kc}")
                for qi in range(2):
                    nc.tensor.matmul(
                        sct[:, qi * 512 : (qi + 1) * 512],
                        kprojT[:, kc * 128 : (kc + 1) * 128],
                        qprojT[:, qi * 512 : (qi + 1) * 512],
                        start=True, stop=True,
                    )
                et = e_pool.tile([128, 1024], bf16, tag="e", name=f"e0_{kc}")
                nc.scalar.activation(et[:], sct[:], AF.Exp)
                e_tiles[(0, kc)] = et

            # ---- V projection + vp assembly ----
            psv = ps_pool.tile([128, 1024], f32, tag="ps", name="psv")
            proj(psv, 2 * HC, vt_sb, 0, KH, 1024)
            vprojT = proj_pool.tile([D, KH], f32, tag="vprojT")
            nc.vector.tensor_scalar_add(vprojT[:], psv[0:D, :], b_sb[:, 2:3])
            pst = ps_pool.tile([128, 1024], f32, tag="ps", name="pst")
            for s in range(KC):
                nc.tensor.transpose(
                    pst[:, s * 128 : s * 128 + D],
                    vprojT[:, s * 128 : (s + 1) * 128],
                    identf[0:D, 0:D],
                )
            for s in range(KC):
                nc.vector.tensor_copy(
                    vp[:, s * (D + 1) : s * (D + 1) + D],
                    pst[:, s * 128 : s * 128 + D],
                )
            nc.vector.memset(vp[:, D :: D + 1], 1.0)

            # ---- Q projection, second half (qn2, qn3) ----
            for qn in (2, 3):
                psq = ps_pool.tile([128, 1024], f32, tag="ps", name=f"psq{qn}")
                proj(psq, 0, qt_sb, qn * 4 * H, 512, 512)
                nc.vector.tensor_scalar_add(
                    qprojT[:, qn * 512 : (qn + 1) * 512], psq[0:D, 0:512],
                    b_sb[:, 0:1],
                )

            acc = acc_pool.tile([D + 1, L], f32, tag="acc")

            def attnv(qnp, kc):
                et = e_tiles[(qnp, kc)]
                for qi in range(2):
                    qn = qnp * 2 + qi
                    nc.tensor.matmul(
                        acc[:, qn * 512 : (qn + 1) * 512],
                        vp[:, kc * (D + 1) : (kc + 1) * (D + 1)],
                        et[:, qi * 512 : (qi + 1) * 512],
                        start=(kc == 0), stop=(kc == KC - 1),
                        skip_group_check=True,
                    )

            # ---- attnV q-half 0 interleaved with scores/exp/attnV q-half 1 ----
            for kc in range(KC):
                attnv(0, kc)
                sct = ps_pool.tile([128, 1024], f32, tag="ps", name=f"sc1_{kc}")
                for qi in range(2):
                    qn = 2 + qi
                    nc.tensor.matmul(
                        sct[:, qi * 512 : (qi + 1) * 512],
                        kprojT[:, kc * 128 : (kc + 1) * 128],
                        qprojT[:, qn * 512 : (qn + 1) * 512],
                        start=True, stop=True,
                    )
                et = e_pool.tile([128, 1024], bf16, tag="e", name=f"e1_{kc}")
                nc.scalar.activation(et[:], sct[:], AF.Exp)
                e_tiles[(1, kc)] = et
                if kc > 0:
                    attnv(1, kc - 1)
            attnv(1, KC - 1)

            # ---- epilogue per q-half: transpose [65, 1024] -> q-natural ----
            for qnp in range(2):
                for qn in (qnp * 2, qnp * 2 + 1):
                    nc.vector.tensor_copy(
                        outT_sb[:, qn * 512 : (qn + 1) * 512],
                        acc[:, qn * 512 : (qn + 1) * 512],
                    )
                eps = ps_pool.tile([128, 1024], f32, tag="ps", name=f"eps{qnp}")
                for c in range(8):
                    qc = qnp * 8 + c
                    nc.tensor.transpose(
                        eps[:, c * 128 : c * 128 + (D + 1)],
                        outT_sb[:, qc * 128 : (qc + 1) * 128],
                        identf[0 : D + 1, 0 : D + 1],
                    )
                for c in range(8):
                    qc = qnp * 8 + c
                    nc.vector.tensor_copy(
                        out_sb[:, qc * (D + 1) : (qc + 1) * (D + 1)],
                        eps[:, c * 128 : c * 128 + (D + 1)],
                    )
                nc.sync.dma_start(
                    out=out_d[:, qnp * 8 * (D + 1) : (qnp + 1) * 8 * (D + 1)],
                    in_=out_sb[:, qnp * 8 * (D + 1) : (qnp + 1) * 8 * (D + 1)],
                )

    nc.compile()
    return nc


_NC_CACHE = None


def _get_nc():
    global _NC_CACHE
    if _NC_CACHE is None:
        _NC_CACHE = build_bass()
    return _NC_CACHE


def _make_in_maps(inputs):
    bf16 = ml_dtypes.bfloat16
    q = np.asarray(inputs["query"], np.float32)
    k = np.asarray(inputs["key"], np.float32)
    v = np.asarray(inputs["value"], np.float32)
    Wq = np.asarray(inputs["Wq"], np.float32) * 0.125
    bq = np.asarray(inputs["bq"], np.float32) * 0.125
    Wk = np.asarray(inputs["Wk"], np.float32)
    bk = np.asarray(inputs["bk"], np.float32)
    Wv = np.asarray(inputs["Wv"], np.float32)
    bv = np.asarray(inputs["bv"], np.float32)

    def packw(W):  # [1024, 64] -> [128, 8*64], hc-major per partition
        return W.reshape(HC, 128, D).transpose(1, 0, 2).reshape(128, HC * D)

    wcat = np.concatenate([packw(Wq), packw(Wk), packw(Wv)], axis=1).astype(bf16)
    bcat = np.stack([bq, bk, bv], axis=1).astype(np.float32)

    def tr(x):  # [S, 1024] -> [128, 8*S]: [p, hc*S + s] = x[s, hc*128+p]
        S = x.shape[0]
        return np.ascontiguousarray(
            x.reshape(S, HC, 128).transpose(2, 1, 0)
        ).reshape(128, HC * S).astype(bf16)

    in_maps = []
    for c in range(NCORES):
        b, j = divmod(c, 2)
        qb = q[b]  # [2048, 1024]
        # [p, qn*4096 + hc*512 + s] = qb[qn*512+s, hc*128+p]
        qT = np.ascontiguousarray(
            qb.reshape(4, 512, HC, 128).transpose(3, 0, 2, 1)
        ).reshape(128, 16 * H).astype(bf16)
        kT = tr(k[b, j * KH : (j + 1) * KH])
        vT = tr(v[b, j * KH : (j + 1) * KH])
        in_maps.append({"qt": qT, "kt": kT, "vt": vT, "w": wcat, "b": bcat})
    return in_maps


def kernel(query, key, value, Wq, bq, Wk, bk, Wv, bv):
    from concourse.bass_utils import run_bass_kernel_spmd

    in_maps = _make_in_maps(
        dict(query=query, key=key, value=value, Wq=Wq, bq=bq, Wk=Wk, bk=bk,
             Wv=Wv, bv=bv)
    )
    nc = _get_nc()
    try:
        res = run_bass_kernel_spmd(nc, in_maps, list(range(NCORES)))
    except Exception:
        res = run_bass_kernel_spmd(nc, in_maps, list(range(NCORES)))
    out = np.empty((N, L, D), np.float32)
    for b in range(N):
        # [p, qc, j]: row qc*128+p holds [num(64), den]
        o0 = np.asarray(res.results[2 * b]["out"], np.float32).reshape(128, 16, D + 1)
        o1 = np.asarray(res.results[2 * b + 1]["out"], np.float32).reshape(128, 16, D + 1)
        o = (o0 + o1).transpose(1, 0, 2).reshape(L, D + 1)
        out[b] = o[:, 0:D] / o[:, D : D + 1]
    return out
